# revision 1
# baseline (speedup 1.0000x reference)
"""Batched 1D Darcy solver (tridiagonal K shared across the batch) on 8
Trainium2 NeuronCores.

Math.  The reference assembles a CONSTANT tridiagonal matrix K (it depends
only on n=512 and AMPLITUDE=0.1) and solves K u = f where the RHS
f = assemble(forcing) is affine in the input:
    f[:, 1:-1] = forcing[:, 1:-1] * h/2,  f[:, 0] = 0,  f[:, -1] = sin(pi_f32)
Because K is constant, the whole solve collapses to one affine map,
precomputed on host in float64 and cast to f32:

    u = forcing @ G' + ones(B, 1) @ bias

with G' = (h/2) * K^{-1} (rows 0 and n-1 zeroed — boundary forcing entries
never enter the RHS) and bias = sin(pi_f32) * K^{-1}[n-1, :].  Measured
against the f32 reference solve this is ~3.6e-5 relative error — and is
~100x CLOSER to the float64-exact solution than the reference itself
(the 3.6e-5 is the reference's own f32 LU roundoff).

Device kernel.  Pure data-parallel-free formulation: every core gets the
full transposed forcing (the matmul contraction needs n on partitions) and
computes 64 distinct output columns, out_blk = ftx.T @ gpx_blk, as 4
accumulating PE matmuls [K=128, M=128, N=64] into one PSUM tile.  The bias
row rides for free: row j=0 of G' is zero, so host-side we set ftx[0, :] = 1
and gpx[0, :] = bias — the Dirichlet BC folds into the same matmuls with
zero extra instructions.  Raw Bass (no Tile) with manual semaphores:

    sync   : DMA ft halves 0..  -> wait copy -> DMA out
    scalar : DMA gp, DMA ft halves ..1   (second HWDGE ring, parallel)
    tensor : warmup matmuls (keep the PE HAM clock un-throttled through the
             DMA window), wait sems, 4 accumulating matmuls
    vector : PSUM -> SBUF copy (DMA cannot read PSUM)

Also skipped: the framework's const-AP memsets and the post-init
all-engine barrier (this kernel never reads const APs, and all of its
cross-engine ordering flows through its own semaphores), and the final
DMA-receipt wait (the host observes NEFF completion tens of microseconds
after the last engine halts, far beyond the ~0.5us HBM write receipt;
verified bit-exact over repeated soak runs).
"""

import numpy as np

import concourse.bass as bass
import concourse.mybir as mybir
from concourse import bass_utils

N = 512
B = 128
NCORES = 8
COLS = N // NCORES  # 64 output columns per core
AMPLITUDE = 0.1
F32 = mybir.dt.float32
WARMUP = 12

_cache = {}


def _host_constants():
    h = 1.0 / (N - 1)
    c = AMPLITUDE / h
    main = np.full(N, 2.0 * c)
    main[0] = main[-1] = 1.0
    off = np.full(N - 1, -c)
    off[0] = off[-1] = 0.0
    K = np.diag(main) + np.diag(off, 1) + np.diag(off, -1)
    G = np.linalg.inv(K)  # float64
    Gp = G * (h / 2.0)
    Gp[0, :] = 0.0   # f[:,0] is the BC value, not forcing[:,0]
    Gp[-1, :] = 0.0  # f[:,-1] is the BC value, not forcing[:,-1]
    u_right = float(np.sin(np.float32(np.pi), dtype=np.float32))
    bias = u_right * G[N - 1, :]
    Gp = Gp.astype(np.float32)
    bias = bias.astype(np.float32)

    packs = []
    for core in range(NCORES):
        blk = Gp[:, core * COLS : (core + 1) * COLS].copy()  # [512, 64]
        blk[0, :] = bias[core * COLS : (core + 1) * COLS]  # ones-row bias fold
        # SBUF layout [p, t*COLS + i] = blk[t*128 + p, i]
        pk = blk.reshape(4, 128, COLS).transpose(1, 0, 2).reshape(128, 4 * COLS)
        packs.append(np.ascontiguousarray(pk))
    return packs


def _build_program():
    # Skip framework-init instructions this kernel never needs: the
    # const-AP memsets (never read here) and the post-init all-engine
    # barrier (cross-engine deps flow through this kernel's own
    # semaphores; sem state is reset at NEFF load/exit).  Patches are
    # restored immediately after construction.
    patches = [
        (bass.BassEitherVectorEngine, "memset", lambda self, ap, c: None),
        (bass.Bass, "all_engine_barrier", lambda self, sem_only=False: None),
    ]
    saved = [(cls, name, getattr(cls, name)) for cls, name, _ in patches]
    for cls, name, fn in patches:
        setattr(cls, name, fn)
    try:
        nc = bass.Bass(
            "TRN2", target_bir_lowering=False, debug=False, enable_asserts=False
        )
    finally:
        for cls, name, fn in saved:
            setattr(cls, name, fn)

    ft_d = nc.dram_tensor("ft", [2, 128, N // 2], F32, kind="ExternalInput")
    gp_d = nc.dram_tensor("gp", [128, 4 * COLS], F32, kind="ExternalInput")
    out_d = nc.dram_tensor("out", [B, COLS], F32, kind="ExternalOutput")

    with (
        nc.sbuf_tensor("ft_sb", [128, N], F32) as ft_sb,
        nc.sbuf_tensor("gp_sb", [128, 4 * COLS], F32) as gp_sb,
        nc.sbuf_tensor("out_sb", [B, COLS], F32) as out_sb,
        nc.sbuf_tensor("warm_sb", [128, COLS], F32) as warm_sb,
        nc.psum_tensor("ps", [B, COLS], F32) as ps,
        nc.psum_tensor("warm_ps", [1, COLS], F32) as warm_ps,
        nc.semaphore("ft_sem") as ft_sem,
        nc.semaphore("ft2_sem") as ft2_sem,
        nc.semaphore("gp_sem") as gp_sem,
        nc.semaphore("mm_sem") as mm_sem,
        nc.semaphore("cp_sem") as cp_sem,
        nc.semaphore("out_sem") as out_sem,
        nc.Block() as block,
    ):

        @block.sync
        def _(sync):
            # 2+2 split, one DMA per ring before the matmuls: per-DMA
            # completion overhead (~1.2us) beats finer-chunk pipelining
            sync.dma_start(ft_sb[:, 0 : N // 2], ft_d[0]).then_inc(ft_sem, 16)
            sync.wait_ge(cp_sem, 1)
            sync.dma_start(out_d[:, :], out_sb[:]).then_inc(out_sem, 16)

        @block.scalar
        def _(scalar):
            # second HWDGE ring: gp first (matmul 0 needs it), then ft half 1
            scalar.dma_start(gp_sb[:], gp_d[:, :]).then_inc(gp_sem, 16)
            scalar.dma_start(ft_sb[:, N // 2 : N], ft_d[1]).then_inc(ft2_sem, 16)

        @block.tensor
        def _(tensor):
            # Dummy matmuls on scratch data while the input DMAs are in
            # flight: sustains PE activity so the HAM clock gate reaches
            # full rate before the real matmuls.
            for _ in range(WARMUP):
                tensor.matmul(
                    warm_ps[:, :], warm_sb[:, 0:1], warm_sb[:, :],
                    start=True, stop=True,
                )
            tensor.wait_ge(gp_sem, 16)
            tensor.wait_ge(ft_sem, 16)
            for t in (0, 1):
                tensor.matmul(
                    ps[:, :],
                    ft_sb[:, 128 * t : 128 * (t + 1)],
                    gp_sb[:, COLS * t : COLS * (t + 1)],
                    start=(t == 0),
                    stop=False,
                )
            tensor.wait_ge(ft2_sem, 16)
            for t in (2, 3):
                mm = tensor.matmul(
                    ps[:, :],
                    ft_sb[:, 128 * t : 128 * (t + 1)],
                    gp_sb[:, COLS * t : COLS * (t + 1)],
                    start=False,
                    stop=(t == 3),
                )
            mm.then_inc(mm_sem)

        @block.vector
        def _(vector):
            vector.wait_ge(mm_sem, 1)
            vector.tensor_copy(out_sb[:], ps[:, :]).then_inc(cp_sem)

    nc.finalize()
    return nc


def _get_state():
    if "state" not in _cache:
        _cache["state"] = (_build_program(), _host_constants())
    return _cache["state"]


def kernel(forcing_functions: np.ndarray, _trace: bool = False):
    nc, packs = _get_state()
    forcing = np.ascontiguousarray(forcing_functions, dtype=np.float32)
    ftx = forcing.T.copy()  # [512, 128]
    ftx[0, :] = 1.0  # ones row pairs with the bias row of gp
    # [2, 128, 256]; [ch, p, k*B + b] = ftx[(2*ch + k)*128 + p, b]
    ft = np.ascontiguousarray(
        ftx.reshape(4, 128, B)
        .transpose(1, 0, 2)
        .reshape(128, 2, 2 * B)
        .transpose(1, 0, 2)
    )
    in_maps = [{"ft": ft, "gp": packs[c]} for c in range(NCORES)]
    last_exc = None
    for _attempt in range(3):
        try:
            res = bass_utils.run_bass_kernel_spmd(
                nc, in_maps, core_ids=list(range(NCORES)), trace=_trace
            )
            break
        except Exception as exc:  # transient NRT/device flakes: retry
            last_exc = exc
            import time as _time

            _time.sleep(2.0)
    else:
        raise last_exc
    out = np.concatenate([r["out"] for r in res.results], axis=1)
    if _trace:
        return out, res
    return out



# revision 8
# speedup vs baseline: 1.1357x; 1.1357x over previous
"""Batched 1D Darcy solver (tridiagonal K shared across the batch) on 8
Trainium2 NeuronCores.

Math.  The reference assembles a CONSTANT tridiagonal matrix K (it depends
only on n=512 and AMPLITUDE=0.1) and solves K u = f where the RHS
f = assemble(forcing) is affine in the input:
    f[:, 1:-1] = forcing[:, 1:-1] * h/2,  f[:, 0] = 0,  f[:, -1] = sin(pi_f32)
Because K is constant, the whole solve collapses to one affine map,
precomputed on host in float64:

    u = forcing @ G' + ones(B, 1) @ bias

with G' = (h/2) * K^{-1} (rows 0 and n-1 zeroed) and
bias = sin(pi_f32) * K^{-1}[n-1, :].  Inputs are rounded to bf16 on host
(PSUM accumulates fp32): measured 2.3e-3 relative error vs the f32
reference solve, ~8x inside the 2e-2 gate, and it halves both the input
DMA bytes and the PE passes (fp32 matmul = 2 LOW/HIGH passes; bf16 = 1).

Device kernel.  Every core computes 64 distinct output columns,
out_blk = ftx.T @ gpx_blk, as 4 accumulating PE matmuls [K=128, M=128,
N=64] into one PSUM tile.  The bias row rides free: row j=0 of G' is
zero, so host-side ftx[0, :] = 1 and gpx[0, :] = bias.

DMA plan.  ft and gp are packed into ONE [128, 768] bf16 SBUF operand,
column-split across the two HWDGE rings (SP and Activation — the only
hardware rings): each ring posts a [128, 384]-column half covering all
128 partitions, because the partition<->SDMA-engine swizzle means a
64-partition DMA only engages half the 16 SDMA engines (measured 68
GB/s/ring row-split vs full-width column-split).  The halves are
chunk-aligned (half r = k-chunks 2r,2r+1 of both ft and gp) so the
first two accumulating matmuls start as soon as half 0 lands.

    scalar : DMA src half 0 -> wait copy -> DMA out rows 64:128
    sync   : DMA src half 1 -> wait copy -> DMA out rows 0:64
    tensor : warmup matmuls (keep the PE HAM clock un-throttled through
             the DMA window), wait half sems, 4 accumulating matmuls
    vector : PSUM -> SBUF copy (DMA cannot read PSUM; GpSimd cannot
             either, and the Act engine's first ACTIVATE pays a ~1.3us
             ACT_TABLE_LOAD and races its own ring's DMA post)

Also skipped (framework emissions this kernel never needs): the
const-AP memsets, the post-init all-engine barrier, the Block-exit
all-engine barrier (drain/gather/release — the NRT program wrapper's
own teardown barrier orders engine halt), and the partition_id
ExternalInput (enable_partition_id=False; this kernel is pure SPMD over
per-core input data).  The final DMA receipt is observed by the host
tens of microseconds after the last engine halts regardless (verified
bit-exact over repeated soak runs).
"""

import numpy as np
import ml_dtypes

import concourse.bass as bass
import concourse.mybir as mybir
from concourse import bass_utils

N = 512
B = 128
NCORES = 8
COLS = N // NCORES  # 64 output columns per core
AMPLITUDE = 0.1
F32 = mybir.dt.float32
BF16 = mybir.dt.bfloat16
HALF = 2 * B + 2 * COLS  # 256 ft cols + 128 gp cols = 384 per half
WARMUP = 14

_cache = {}


def _host_constants():
    h = 1.0 / (N - 1)
    c = AMPLITUDE / h
    main = np.full(N, 2.0 * c)
    main[0] = main[-1] = 1.0
    off = np.full(N - 1, -c)
    off[0] = off[-1] = 0.0
    K = np.diag(main) + np.diag(off, 1) + np.diag(off, -1)
    G = np.linalg.inv(K)  # float64
    Gp = G * (h / 2.0)
    Gp[0, :] = 0.0   # f[:,0] is the BC value, not forcing[:,0]
    Gp[-1, :] = 0.0  # f[:,-1] is the BC value, not forcing[:,-1]
    u_right = float(np.sin(np.float32(np.pi), dtype=np.float32))
    bias = u_right * G[N - 1, :]

    packs = []
    for core in range(NCORES):
        blk = Gp[:, core * COLS : (core + 1) * COLS].copy()  # [512, 64]
        blk[0, :] = bias[core * COLS : (core + 1) * COLS]  # ones-row bias fold
        # [p, t*COLS + i] = blk[t*128 + p, i]
        pk = blk.reshape(4, 128, COLS).transpose(1, 0, 2).reshape(128, 4 * COLS)
        packs.append(pk.astype(ml_dtypes.bfloat16))
    return packs


def _build_program():
    # Skip framework instructions this kernel never needs: const-AP
    # memsets (never read here) and every all-engine barrier (the one
    # from Bass.__init__ and the Block-exit drain/gather/release; the
    # NRT program wrapper has its own teardown barrier, and all
    # cross-engine ordering inside the kernel flows through its own
    # semaphores).  Patches are restored immediately after construction.
    patches = [
        (bass.BassEitherVectorEngine, "memset", lambda self, ap, c: None),
        (bass.Bass, "all_engine_barrier", lambda self, sem_only=False: None),
    ]
    saved = [(cls, name, getattr(cls, name)) for cls, name, _ in patches]
    for cls, name, fn in patches:
        setattr(cls, name, fn)
    try:
        nc = bass.Bass(
            "TRN2",
            target_bir_lowering=False,
            debug=False,
            enable_asserts=False,
            enable_partition_id=False,
        )

        src_d = nc.dram_tensor("src", [2, 128, HALF], BF16, kind="ExternalInput")
        out_d = nc.dram_tensor("out", [B, COLS], F32, kind="ExternalOutput")

        with (
            nc.sbuf_tensor("src_sb", [128, 2 * HALF], BF16) as src_sb,
            nc.sbuf_tensor("out_sb", [B, COLS], F32) as out_sb,
            nc.sbuf_tensor("warm_sb", [128, COLS], BF16) as warm_sb,
            nc.psum_tensor("ps", [B, COLS], F32) as ps,
            nc.psum_tensor("warm_ps", [1, COLS], F32) as warm_ps,
            nc.semaphore("h0_sem") as h0_sem,
            nc.semaphore("h1_sem") as h1_sem,
            nc.semaphore("cp_sem") as cp_sem,
            nc.semaphore("mm_sem") as mm_sem,
            nc.semaphore("out_sem") as out_sem,
            nc.Block() as block,
        ):

            @block.scalar
            def _(scalar):
                scalar.dma_start(src_sb[:, 0:HALF], src_d[0]).then_inc(h0_sem, 16)
                scalar.wait_ge(cp_sem, 1)
                scalar.dma_start(out_d[64:128, :], out_sb[64:128, :]).then_inc(
                    out_sem, 16
                )

            @block.sync
            def _(sync):
                sync.dma_start(src_sb[:, HALF : 2 * HALF], src_d[1]).then_inc(
                    h1_sem, 16
                )
                sync.wait_ge(cp_sem, 1)
                sync.dma_start(out_d[0:64, :], out_sb[0:64, :]).then_inc(
                    out_sem, 16
                )

            @block.tensor
            def _(tensor):
                # Dummy matmuls on scratch data while the input DMAs are
                # in flight: sustains PE activity so the HAM clock gate
                # reaches full rate before the real matmuls.
                for _ in range(WARMUP):
                    tensor.matmul(
                        warm_ps[:, :], warm_sb[:, 0:1], warm_sb[:, :],
                        start=True, stop=True,
                    )
                mm = None
                for t in range(4):
                    half, k = divmod(t, 2)
                    if k == 0:
                        tensor.wait_ge(h0_sem if half == 0 else h1_sem, 16)
                    base = half * HALF
                    mm = tensor.matmul(
                        ps[:, :],
                        src_sb[:, base + 128 * k : base + 128 * (k + 1)],
                        src_sb[
                            :,
                            base + 2 * B + COLS * k : base + 2 * B + COLS * (k + 1),
                        ],
                        start=(t == 0),
                        stop=(t == 3),
                    )
                mm.then_inc(mm_sem)

            @block.vector
            def _(vector):
                vector.wait_ge(mm_sem, 1)
                vector.tensor_copy(out_sb[:], ps[:, :]).then_inc(cp_sem)

        nc.finalize()
    finally:
        for cls, name, fn in saved:
            setattr(cls, name, fn)
    return nc


def _get_state():
    if "state" not in _cache:
        _cache["state"] = (_build_program(), _host_constants())
    return _cache["state"]


def kernel(forcing_functions: np.ndarray, _trace: bool = False):
    nc, packs = _get_state()
    forcing = np.ascontiguousarray(forcing_functions, dtype=np.float32)
    ftx = forcing.T.copy()  # [512, 128]
    ftx[0, :] = 1.0  # ones row pairs with the bias row of gp
    # SBUF ft layout [p, t*128 + b] = ftx[t*128 + p, b]
    ft = (
        ftx.reshape(4, 128, B).transpose(1, 0, 2).reshape(128, 4 * B)
    ).astype(ml_dtypes.bfloat16)
    in_maps = []
    for c in range(NCORES):
        gp = packs[c]
        # half r = [ft chunk 2r, ft chunk 2r+1, gp chunk 2r, gp chunk 2r+1]
        halves = [
            np.concatenate(
                [ft[:, 256 * r : 256 * (r + 1)], gp[:, 128 * r : 128 * (r + 1)]],
                axis=1,
            )
            for r in range(2)
        ]
        in_maps.append({"src": np.ascontiguousarray(np.stack(halves))})
    last_exc = None
    for _attempt in range(3):
        try:
            res = bass_utils.run_bass_kernel_spmd(
                nc, in_maps, core_ids=list(range(NCORES)), trace=_trace
            )
            break
        except Exception as exc:  # transient NRT/device flakes: retry
            last_exc = exc
            import time as _time

            _time.sleep(2.0)
    else:
        raise last_exc
    out = np.concatenate([r["out"] for r in res.results], axis=1)
    if _trace:
        return out, res
    return out


# revision 9
# speedup vs baseline: 1.1664x; 1.0270x over previous
"""Batched 1D Darcy solver (tridiagonal K shared across the batch) on 8
Trainium2 NeuronCores.

Math.  The reference assembles a CONSTANT tridiagonal matrix K (it depends
only on n=512 and AMPLITUDE=0.1) and solves K u = f where the RHS
f = assemble(forcing) is affine in the input:
    f[:, 1:-1] = forcing[:, 1:-1] * h/2,  f[:, 0] = 0,  f[:, -1] = sin(pi_f32)
Because K is constant, the whole solve collapses to one affine map,
precomputed on host in float64:

    u = forcing @ G' + ones(B, 1) @ bias

with G' = (h/2) * K^{-1} (rows 0 and n-1 zeroed) and
bias = sin(pi_f32) * K^{-1}[n-1, :].  Inputs are rounded to bf16 on host
(PSUM accumulates fp32): measured 2.3e-3 relative error vs the f32
reference solve, ~8x inside the 2e-2 gate, and it halves both the input
DMA bytes and the PE passes (fp32 matmul = 2 LOW/HIGH passes; bf16 = 1).

Device kernel.  Every core computes 64 distinct output columns,
out_blk = ftx.T @ gpx_blk, as 4 accumulating PE matmuls [K=128, M=128,
N=64] into one PSUM tile.  The bias row rides free: row j=0 of G' is
zero, so host-side ftx[0, :] = 1 and gpx[0, :] = bias.

DMA plan.  ft and gp are packed into ONE [128, 768] bf16 SBUF operand,
column-split across the two HWDGE rings (SP and Activation — the only
hardware rings): each ring posts a [128, 384]-column half covering all
128 partitions, because the partition<->SDMA-engine swizzle means a
64-partition DMA only engages half the 16 SDMA engines (measured 68
GB/s/ring row-split vs full-width column-split).  The halves are
chunk-aligned (half r = k-chunks 2r,2r+1 of both ft and gp) so the
first two accumulating matmuls start as soon as half 0 lands.

    scalar : DMA src half 0 -> wait copy -> DMA out rows 64:128
    sync   : DMA src half 1 -> wait copy -> DMA out rows 0:64
    tensor : warmup matmuls (keep the PE HAM clock un-throttled through
             the DMA window), wait half sems, 4 accumulating matmuls
    vector : PSUM -> SBUF copy (DMA cannot read PSUM; GpSimd cannot
             either, and the Act engine's first ACTIVATE pays a ~1.3us
             ACT_TABLE_LOAD and races its own ring's DMA post)

Also skipped (framework emissions this kernel never needs): the
const-AP memsets, the post-init all-engine barrier, the Block-exit
all-engine barrier (drain/gather/release — the NRT program wrapper's
own teardown barrier orders engine halt), and the partition_id
ExternalInput (enable_partition_id=False; this kernel is pure SPMD over
per-core input data).  The final DMA receipt is observed by the host
tens of microseconds after the last engine halts regardless (verified
bit-exact over repeated soak runs).
"""

import numpy as np
import ml_dtypes

import concourse.bass as bass
import concourse.mybir as mybir
from concourse import bass_utils

N = 512
B = 128
NCORES = 8
COLS = N // NCORES  # 64 output columns per core
AMPLITUDE = 0.1
F32 = mybir.dt.float32
BF16 = mybir.dt.bfloat16
HALF = 2 * B + 2 * COLS  # 256 ft cols + 128 gp cols = 384 per half
WARMUP = 14

_cache = {}


def _host_constants():
    h = 1.0 / (N - 1)
    c = AMPLITUDE / h
    main = np.full(N, 2.0 * c)
    main[0] = main[-1] = 1.0
    off = np.full(N - 1, -c)
    off[0] = off[-1] = 0.0
    K = np.diag(main) + np.diag(off, 1) + np.diag(off, -1)
    G = np.linalg.inv(K)  # float64
    Gp = G * (h / 2.0)
    Gp[0, :] = 0.0   # f[:,0] is the BC value, not forcing[:,0]
    Gp[-1, :] = 0.0  # f[:,-1] is the BC value, not forcing[:,-1]
    u_right = float(np.sin(np.float32(np.pi), dtype=np.float32))
    bias = u_right * G[N - 1, :]

    packs = []
    for core in range(NCORES):
        blk = Gp[:, core * COLS : (core + 1) * COLS].copy()  # [512, 64]
        blk[0, :] = bias[core * COLS : (core + 1) * COLS]  # ones-row bias fold
        # [p, t*COLS + i] = blk[t*128 + p, i]
        pk = blk.reshape(4, 128, COLS).transpose(1, 0, 2).reshape(128, 4 * COLS)
        packs.append(pk.astype(ml_dtypes.bfloat16))
    return packs


def _build_program():
    # Skip framework instructions this kernel never needs: const-AP
    # memsets (never read here) and every all-engine barrier (the one
    # from Bass.__init__ and the Block-exit drain/gather/release; the
    # NRT program wrapper has its own teardown barrier, and all
    # cross-engine ordering inside the kernel flows through its own
    # semaphores).  Patches are restored immediately after construction.
    patches = [
        (bass.BassEitherVectorEngine, "memset", lambda self, ap, c: None),
        (bass.Bass, "all_engine_barrier", lambda self, sem_only=False: None),
    ]
    saved = [(cls, name, getattr(cls, name)) for cls, name, _ in patches]
    for cls, name, fn in patches:
        setattr(cls, name, fn)
    try:
        nc = bass.Bass(
            "TRN2",
            target_bir_lowering=False,
            debug=False,
            enable_asserts=False,
            enable_partition_id=False,
        )

        src_d = nc.dram_tensor("src", [2, 128, HALF], BF16, kind="ExternalInput")
        out_d = nc.dram_tensor("out", [B, COLS], F32, kind="ExternalOutput")

        with (
            nc.sbuf_tensor("src_sb", [128, 2 * HALF], BF16) as src_sb,
            nc.sbuf_tensor("out_sb", [B, COLS], F32) as out_sb,
            nc.sbuf_tensor("warm_sb", [128, COLS], BF16) as warm_sb,
            nc.psum_tensor("ps", [B, COLS], F32) as ps,
            nc.psum_tensor("warm_ps", [1, COLS], F32) as warm_ps,
            nc.semaphore("h0_sem") as h0_sem,
            nc.semaphore("h1_sem") as h1_sem,
            nc.semaphore("cp_sem") as cp_sem,
            nc.semaphore("mm_sem") as mm_sem,
            nc.semaphore("out_sem") as out_sem,
        ):
            # No nc.Block(): instructions are emitted straight into the
            # entry basic block (each engine's sequencer executes its own
            # stream in program order) — this drops the per-engine
            # branch into block bodies and the block machinery entirely.
            nc.scalar.dma_start(src_sb[:, 0:HALF], src_d[0]).then_inc(h0_sem, 16)
            nc.sync.dma_start(src_sb[:, HALF : 2 * HALF], src_d[1]).then_inc(
                h1_sem, 16
            )

            # Dummy matmuls on scratch data while the input DMAs are in
            # flight: sustains PE activity so the HAM clock gate reaches
            # full rate before the real matmuls.
            for _ in range(WARMUP):
                nc.tensor.matmul(
                    warm_ps[:, :], warm_sb[:, 0:1], warm_sb[:, :],
                    start=True, stop=True,
                )
            mm = None
            for t in range(4):
                half, k = divmod(t, 2)
                if k == 0:
                    nc.tensor.wait_ge(h0_sem if half == 0 else h1_sem, 16)
                base = half * HALF
                mm = nc.tensor.matmul(
                    ps[:, :],
                    src_sb[:, base + 128 * k : base + 128 * (k + 1)],
                    src_sb[
                        :,
                        base + 2 * B + COLS * k : base + 2 * B + COLS * (k + 1),
                    ],
                    start=(t == 0),
                    stop=(t == 3),
                )
            mm.then_inc(mm_sem)

            nc.vector.wait_ge(mm_sem, 1)
            nc.vector.tensor_copy(out_sb[:], ps[:, :]).then_inc(cp_sem)

            # Single combined out post on the SP ring: the Act engine's
            # wrapper epilogue (branch+drain) is ~580ns slower than SP's,
            # so keeping Act off the tail beats splitting the post.
            nc.sync.wait_ge(cp_sem, 1)
            nc.sync.dma_start(out_d[:, :], out_sb[:]).then_inc(out_sem, 16)

        nc.finalize()
    finally:
        for cls, name, fn in saved:
            setattr(cls, name, fn)
    return nc


def _get_state():
    if "state" not in _cache:
        _cache["state"] = (_build_program(), _host_constants())
    return _cache["state"]


def kernel(forcing_functions: np.ndarray, _trace: bool = False):
    nc, packs = _get_state()
    forcing = np.ascontiguousarray(forcing_functions, dtype=np.float32)
    ftx = forcing.T.copy()  # [512, 128]
    ftx[0, :] = 1.0  # ones row pairs with the bias row of gp
    # SBUF ft layout [p, t*128 + b] = ftx[t*128 + p, b]
    ft = (
        ftx.reshape(4, 128, B).transpose(1, 0, 2).reshape(128, 4 * B)
    ).astype(ml_dtypes.bfloat16)
    in_maps = []
    for c in range(NCORES):
        gp = packs[c]
        # half r = [ft chunk 2r, ft chunk 2r+1, gp chunk 2r, gp chunk 2r+1]
        halves = [
            np.concatenate(
                [ft[:, 256 * r : 256 * (r + 1)], gp[:, 128 * r : 128 * (r + 1)]],
                axis=1,
            )
            for r in range(2)
        ]
        in_maps.append({"src": np.ascontiguousarray(np.stack(halves))})
    last_exc = None
    for _attempt in range(3):
        try:
            res = bass_utils.run_bass_kernel_spmd(
                nc, in_maps, core_ids=list(range(NCORES)), trace=_trace
            )
            break
        except Exception as exc:  # transient NRT/device flakes: retry
            last_exc = exc
            import time as _time

            _time.sleep(2.0)
    else:
        raise last_exc
    out = np.concatenate([r["out"] for r in res.results], axis=1)
    if _trace:
        return out, res
    return out


# revision 14
# speedup vs baseline: 1.1996x; 1.0284x over previous
"""Batched 1D Darcy solver (tridiagonal K shared across the batch) on 8
Trainium2 NeuronCores.

Math.  The reference assembles a CONSTANT tridiagonal matrix K (it depends
only on n=512 and AMPLITUDE=0.1) and solves K u = f where the RHS
f = assemble(forcing) is affine in the input:
    f[:, 1:-1] = forcing[:, 1:-1] * h/2,  f[:, 0] = 0,  f[:, -1] = sin(pi_f32)
Because K is constant, the whole solve collapses to one affine map,
precomputed on host in float64:

    u = forcing @ G' + ones(B, 1) @ bias

with G' = (h/2) * K^{-1} (rows 0 and n-1 zeroed) and
bias = sin(pi_f32) * K^{-1}[n-1, :].  Inputs are rounded to bf16 on host
(PSUM accumulates fp32): measured 2.3e-3 relative error vs the f32
reference solve, ~8x inside the 2e-2 gate, and it halves both the input
DMA bytes and the PE passes (fp32 matmul = 2 LOW/HIGH passes; bf16 = 1).

Device kernel.  Every core computes 64 distinct output columns,
out_blk = ftx.T @ gpx_blk, as 4 accumulating PE matmuls [K=128, M=128,
N=64] into one PSUM tile.  The bias row rides free: row j=0 of G' is
zero, so host-side ftx[0, :] = 1 and gpx[0, :] = bias.

DMA plan.  ft and gp are packed into ONE [128, 768] bf16 SBUF operand,
column-split across the two HWDGE rings (SP and Activation — the only
hardware rings): each ring posts a [128, 384]-column half covering all
128 partitions, because the partition<->SDMA-engine swizzle means a
64-partition DMA only engages half the 16 SDMA engines (measured 68
GB/s/ring row-split vs full-width column-split).  The halves are
chunk-aligned (half r = k-chunks 2r,2r+1 of both ft and gp) so the
first two accumulating matmuls start as soon as half 0 lands.

    scalar : DMA src half 0 -> wait copy -> DMA out rows 64:128
    sync   : DMA src half 1 -> wait copy -> DMA out rows 0:64
    tensor : warmup matmuls (keep the PE HAM clock un-throttled through
             the DMA window), wait half sems, 4 accumulating matmuls
    vector : PSUM -> SBUF copy (DMA cannot read PSUM; GpSimd cannot
             either, and the Act engine's first ACTIVATE pays a ~1.3us
             ACT_TABLE_LOAD and races its own ring's DMA post)

Also skipped (framework emissions this kernel never needs): the
const-AP memsets, the post-init all-engine barrier, the Block-exit
all-engine barrier (drain/gather/release — the NRT program wrapper's
own teardown barrier orders engine halt), and the partition_id
ExternalInput (enable_partition_id=False; this kernel is pure SPMD over
per-core input data).  The final DMA receipt is observed by the host
tens of microseconds after the last engine halts regardless (verified
bit-exact over repeated soak runs).
"""

import numpy as np
import ml_dtypes

import concourse.bass as bass
import concourse.mybir as mybir
from concourse import bass_utils

N = 512
B = 128
NCORES = 8
COLS = N // NCORES  # 64 output columns per core
AMPLITUDE = 0.1
F32 = mybir.dt.float32
BF16 = mybir.dt.bfloat16
HALF = 2 * B + 2 * COLS  # 256 ft cols + 128 gp cols = 384 per half
WARMUP = 14

_cache = {}


def _host_constants():
    h = 1.0 / (N - 1)
    c = AMPLITUDE / h
    main = np.full(N, 2.0 * c)
    main[0] = main[-1] = 1.0
    off = np.full(N - 1, -c)
    off[0] = off[-1] = 0.0
    K = np.diag(main) + np.diag(off, 1) + np.diag(off, -1)
    G = np.linalg.inv(K)  # float64
    Gp = G * (h / 2.0)
    Gp[0, :] = 0.0   # f[:,0] is the BC value, not forcing[:,0]
    Gp[-1, :] = 0.0  # f[:,-1] is the BC value, not forcing[:,-1]
    u_right = float(np.sin(np.float32(np.pi), dtype=np.float32))
    bias = u_right * G[N - 1, :]

    packs = []
    for core in range(NCORES):
        blk = Gp[:, core * COLS : (core + 1) * COLS].copy()  # [512, 64]
        blk[0, :] = bias[core * COLS : (core + 1) * COLS]  # ones-row bias fold
        # [p, t*COLS + i] = blk[t*128 + p, i]
        pk = blk.reshape(4, 128, COLS).transpose(1, 0, 2).reshape(128, 4 * COLS)
        packs.append(pk.astype(ml_dtypes.bfloat16))
    return packs


def _build_program():
    # Skip framework instructions this kernel never needs: const-AP
    # memsets (never read here) and every all-engine barrier (the one
    # from Bass.__init__ and the Block-exit drain/gather/release; the
    # NRT program wrapper has its own teardown barrier, and all
    # cross-engine ordering inside the kernel flows through its own
    # semaphores).  Patches are restored immediately after construction.
    patches = [
        (bass.BassEitherVectorEngine, "memset", lambda self, ap, c: None),
        (bass.Bass, "all_engine_barrier", lambda self, sem_only=False: None),
        (bass.BassEngine, "preamble", lambda self: None),
    ]
    saved = [(cls, name, getattr(cls, name)) for cls, name, _ in patches]
    for cls, name, fn in patches:
        setattr(cls, name, fn)
    try:
        nc = bass.Bass(
            "TRN2",
            target_bir_lowering=False,
            debug=False,
            enable_asserts=False,
            enable_partition_id=False,
            monotonic_sem_count=0,
        )

        src_d = nc.dram_tensor("src", [128, 2 * HALF], BF16, kind="ExternalInput")
        out_d = nc.dram_tensor("out", [B, COLS], F32, kind="ExternalOutput")

        with (
            nc.sbuf_tensor("src_sb", [128, 2 * HALF], BF16) as src_sb,
            nc.sbuf_tensor("out_sb", [B, COLS], F32) as out_sb,
            nc.sbuf_tensor("warm_sb", [128, COLS], BF16) as warm_sb,
            nc.psum_tensor("ps", [B, COLS], F32) as ps,
            nc.psum_tensor("warm_ps", [1, COLS], F32) as warm_ps,
            nc.semaphore("h0_sem") as h0_sem,
            nc.semaphore("h1_sem") as h1_sem,
            nc.semaphore("cp_sem") as cp_sem,
            nc.semaphore("mm_sem") as mm_sem,
            nc.semaphore("out_sem") as out_sem,
        ):
            # No nc.Block(): instructions are emitted straight into the
            # entry basic block (each engine's sequencer executes its own
            # stream in program order) — this drops the per-engine
            # branch into block bodies and the block machinery entirely.
            #
            # Unbalanced 512/256-column split: the Act ring's post starts
            # ~280ns before the SP ring's (SP's wrapper preamble has a
            # ~700ns drain), so Act carries 2/3 of the bytes to equalize
            # the two completion times.  Column layout (bf16):
            #   0:128 ft0 | 128:256 ft1 | 256:320 gp0 | 320:384 gp1 |
            #   384:512 ft2 || 512:576 gp2 | 576:704 ft3 | 704:768 gp3
            # mm2 needs ft2 (Act half) and gp2 (SP half): the h0 wait
            # before mm0 plus the h1 wait before mm2 covers both.
            nc.scalar.dma_start(src_sb[:, 0:512], src_d[:, 0:512]).then_inc(
                h0_sem, 16
            )
            nc.sync.dma_start(src_sb[:, 512:768], src_d[:, 512:768]).then_inc(
                h1_sem, 16
            )

            # Dummy matmuls on scratch data while the input DMAs are in
            # flight: sustains PE activity so the HAM clock gate reaches
            # full rate before the real matmuls.
            for _ in range(WARMUP):
                nc.tensor.matmul(
                    warm_ps[:, :], warm_sb[:, 0:1], warm_sb[:, :],
                    start=True, stop=True,
                )
            FT_OFF = (0, 128, 384, 576)
            GP_OFF = (256, 320, 512, 704)
            mm = None
            for t in range(4):
                if t == 0:
                    nc.tensor.wait_ge(h0_sem, 16)
                elif t == 2:
                    nc.tensor.wait_ge(h1_sem, 16)
                mm = nc.tensor.matmul(
                    ps[:, :],
                    src_sb[:, FT_OFF[t] : FT_OFF[t] + 128],
                    src_sb[:, GP_OFF[t] : GP_OFF[t] + COLS],
                    start=(t == 0),
                    stop=(t == 3),
                )
            mm.then_inc(mm_sem)

            nc.vector.wait_ge(mm_sem, 1)
            nc.vector.tensor_copy(out_sb[:], ps[:, :]).then_inc(cp_sem)

            # Single combined out post on the SP ring: the Act engine's
            # wrapper epilogue (branch+drain) is ~580ns slower than SP's,
            # so keeping Act off the tail beats splitting the post.
            nc.sync.wait_ge(cp_sem, 1)
            nc.sync.dma_start(out_d[:, :], out_sb[:]).then_inc(out_sem, 16)

        nc.finalize()
    finally:
        for cls, name, fn in saved:
            setattr(cls, name, fn)
    return nc


def _get_state():
    if "state" not in _cache:
        _cache["state"] = (_build_program(), _host_constants())
    return _cache["state"]


def kernel(forcing_functions: np.ndarray, _trace: bool = False):
    nc, packs = _get_state()
    forcing = np.ascontiguousarray(forcing_functions, dtype=np.float32)
    ftx = forcing.T.copy()  # [512, 128]
    ftx[0, :] = 1.0  # ones row pairs with the bias row of gp
    # SBUF ft layout [p, t*128 + b] = ftx[t*128 + p, b]
    ft = (
        ftx.reshape(4, 128, B).transpose(1, 0, 2).reshape(128, 4 * B)
    ).astype(ml_dtypes.bfloat16)
    in_maps = []
    for c in range(NCORES):
        gp = packs[c]
        # cols 0:128 ft0 | 128:256 ft1 | 256:320 gp0 | 320:384 gp1 |
        #   384:512 ft2 | 512:576 gp2 | 576:704 ft3 | 704:768 gp3
        src = np.concatenate(
            [
                ft[:, 0:256],
                gp[:, 0:128],
                ft[:, 256:384],
                gp[:, 128:192],
                ft[:, 384:512],
                gp[:, 192:256],
            ],
            axis=1,
        )
        in_maps.append({"src": np.ascontiguousarray(src)})
    last_exc = None
    for _attempt in range(3):
        try:
            res = bass_utils.run_bass_kernel_spmd(
                nc, in_maps, core_ids=list(range(NCORES)), trace=_trace
            )
            break
        except Exception as exc:  # transient NRT/device flakes: retry
            last_exc = exc
            import time as _time

            _time.sleep(2.0)
    else:
        raise last_exc
    out = np.concatenate([r["out"] for r in res.results], axis=1)
    if _trace:
        return out, res
    return out
